# revision 42
# baseline (speedup 1.0000x reference)
"""Trainium2 Bass kernel for the BiDirectionalRNN problem.

Math (matches the fp32 jax reference):
    e = emb[x]                                   # [B, T, 512]
    fwd:  h_t = relu(e_t @ Wf.T + bf + h_{t-1})  # fs[t]
    bwd over reversed e: bs[s]                   # generation order
    xcat = concat_t [fs[t], bs[t]]  -> [B, T*1024]
    h1 = relu(xcat @ W1.T + b1); 4x h = relu(h @ W2.T + b2); out = h @ Wo.T + bo

Strategy:
  * Data-parallel over batch: 1024/8 = 128 samples per NeuronCore.
  * Host folds embedding + input projection weights into per-direction
    tables WfeB = Wf @ emb.T + bf ([97, 512] per dir, fp16) and builds
    the one-hot of x ([97, 2*BL*T] fp8e3 — pure index reformatting). The
    drive terms a = WfeB @ onehot are K=97 matmuls (fp16 x fp8 mixed).
  * ScalarE copies each a-GEMM PSUM block into the scan layout
    [p, b*33 + s] (strided 3D AP), separator column = -60000 (fp16).
  * The 32-step recurrence h = relu(a + h_prev) runs as a DVE
    tensor_tensor_scan per (dir, hid-tile): state=(a add state) max 0,
    fp32 internal state; the separator resets state to 0 between chains.
    The first scan (the pipeline head) is split into q-pair quarters that
    chase the ScalarE PSUM copies, so the copy chain and the scan chain
    pipeline across the two engines.
  * W1 ships pre-scaled by SW=256 in three precisions, keyed to how the
    RNN state magnitude (and hence the error weight h_t*dW) grows with t:
    t<4 as fp8 e4m3 run in DoubleRow perf mode (both operands fp8e4,
    2 k-tiles per instruction at 0.5 cycles/col — a per-j e4m3 copy of
    the t<4 scan slice feeds the stationary side), t in [4,20) as fp8
    e3m4 (moving-operand only: matmul speed keys off the MOVING dtype
    and mixed input dtypes are allowed, so the xcat side stays fp16),
    t>=20 as fp16. 22MB/core instead of 32MB bf16; rel err 1.66e-2 vs
    the 2e-2 gate. Groups are (dir, m)-major, within each j ordered
    fp16-first / DoubleRow-last so the DMA stream runs ahead of PE and
    the end-of-stream critical chain is the smallest group. The GEMM
    accumulates into one PSUM bank; lhsT = strided scan-output views;
    the SW scale rides through the relu chain (relu(s*x) = s*relu(x))
    and is undone by shipping Wo/SW. a-phases pipeline three js ahead.
  * Tail: PE-transpose h1, then 4 x [512,512] + [97,512] in transposed
    (feature-major) layout; per-m [128,128] PSUM tiles drain alternately
    on ScalarE/VectorE so the next layer's k-matmuls chase the drains;
    the output head runs in batch halves on twin PSUM banks. Biases enter
    PSUM via rank-1 matmuls (b1/b2 pre-scaled by SW on host).
  * DMA order follows the critical path: dir-0 WFE + onehot first (the
    first a-matmul gates everything), W1 stream next, dir-1 onehot inside
    the loop at j=1, tail-only W2/Wo after the W1 stream. a-phase PSUM is
    a 7-deep ring + 1 bank for the W1 accumulator; the a-ring is released
    after the GEMM so the tail pools (2+4+2 banks) fit in the 8 banks.
"""

import numpy as np
import ml_dtypes

F16 = np.float16
E3M4 = ml_dtypes.float8_e3m4
F8E4 = ml_dtypes.float8_e4m3
SW = 256.0            # W1 pre-scale; psum is SW-scaled, undone in the head
NT8 = 20              # t < NT8 ships as fp8e3 (RNN states are small early on)

MOD = 97
HID = 512
T = 32
B = 1024
NCORES = 8
BL = B // NCORES          # 128 batch per core
CL = T + 1                # chain length incl. separator column
FREE = BL * CL            # 4224 scan columns per tile
NEG = -60000.0
W1_GRP = 64               # W1 DMA groups of 4 k-chunks (512KB each)

_CACHE: dict = {}


def _build(reps=1):
    import concourse.tile as tile
    from concourse import bacc, mybir

    fp32 = mybir.dt.float32
    fp16 = mybir.dt.float16
    fp8 = mybir.dt.float8e3
    fp8e4 = mybir.dt.float8e4

    nc = bacc.Bacc(
        "TRN2", target_bir_lowering=False, debug=False, num_devices=NCORES
    )

    d = {
        "IDA": nc.dram_tensor("IDA", [128, 129], fp16, kind="ExternalInput").ap(),
        "WFE": nc.dram_tensor("WFE", [MOD, 2 * HID], fp16, kind="ExternalInput").ap(),
        "W1D": nc.dram_tensor("W1D", [8, 128, 2048], fp8e4, kind="ExternalInput").ap(),
        "W1A": nc.dram_tensor("W1A", [W1_GRP // 2, 128, 2048], fp8, kind="ExternalInput").ap(),
        "W1B": nc.dram_tensor("W1B", [W1_GRP * 3 // 8, 128, 2048], fp16, kind="ExternalInput").ap(),
        "W2O": nc.dram_tensor("W2O", [128, 4 * 512 + 4 * MOD], fp16, kind="ExternalInput").ap(),
        "BIA": nc.dram_tensor("BIA", [1, 1121], fp16, kind="ExternalInput").ap(),
        "OHX": nc.dram_tensor("OHX", [MOD, 2 * BL * T], fp8, kind="ExternalInput").ap(),
        "OUT": nc.dram_tensor("OUT", [MOD, BL], fp32, kind="ExternalOutput").ap(),
    }

    with tile.TileContext(nc) as tc:
        for _ in range(reps):
            _emit(tc, d, mybir)

    nc.compile()
    return nc


def _emit(tc, d, mybir):
    nc = tc.nc
    fp32 = mybir.dt.float32
    fp16 = mybir.dt.float16
    fp8 = mybir.dt.float8e3
    fp8e4 = mybir.dt.float8e4
    AF = mybir.ActivationFunctionType
    ALU = mybir.AluOpType

    from contextlib import ExitStack

    with ExitStack() as ctx:
        const = ctx.enter_context(tc.tile_pool(name="const", bufs=1))
        a_pool = ctx.enter_context(tc.tile_pool(name="apool", bufs=3))
        h_pool = ctx.enter_context(tc.tile_pool(name="hpool", bufs=4))
        w1_pool = ctx.enter_context(tc.tile_pool(name="w1pool", bufs=24))
        hp_pool = ctx.enter_context(tc.tile_pool(name="hppool", bufs=3))
        x8_pool = ctx.enter_context(tc.tile_pool(name="x8pool", bufs=4))
        h1_ctx = ExitStack()
        ps_h1 = h1_ctx.enter_context(tc.tile_pool(name="psh1", bufs=1, space="PSUM"))
        a_ctx = ExitStack()
        ps_a = a_ctx.enter_context(tc.tile_pool(name="psa", bufs=7, space="PSUM"))

        # ---- head ----
        # HWDGE descriptor generations serialize (~0.65us each), so DMA
        # instruction order IS the critical path: dir-0 WFE + onehot feed
        # the first a-phase, then the W1 stream; dir-1 onehot ships inside
        # the loop (first needed at j=4)
        ohall = const.tile([MOD, 2 * BL * T], fp8)
        ohsb = [ohall[:, 0:BL * T], ohall[:, BL * T:2 * BL * T]]
        wfe = const.tile([MOD, 2 * HID], fp16)
        nc.sync.dma_start(wfe[:, 0:HID], d["WFE"][:, 0:HID])
        nc.sync.dma_start(ohsb[0][:, 0:1024], d["OHX"][:, 0:1024])
        nc.sync.dma_start(ohsb[0][:, 1024:4096], d["OHX"][:, 1024:4096])
        bia = const.tile([1, 1121], fp16)
        nc.sync.dma_start(bia[:], d["BIA"])
        b1sb = bia[:, 0:512]
        b2r = bia[:, 512:1024]
        bor = bia[:, 1024:1121]
        nc.sync.dma_start(wfe[:, HID:2 * HID], d["WFE"][:, HID:2 * HID])
        w1_pre = {}
        for G in (0, 1):
            w_t = w1_pool.tile([128, 2048], fp16, tag="w_t")
            nc.sync.dma_start(w_t[:], d["W1B"][G])
            w1_pre[G] = w_t
        ida = const.tile([128, 129], fp16)
        nc.sync.dma_start(ida[:], d["IDA"])
        idsb = ida[:, 0:128]
        w2o = const.tile([128, 4 * 512 + 4 * MOD], fp16)
        w2sb = w2o[:, 0:2048]
        wosb = w2o[:, 2048:2048 + 4 * MOD]
        ones = const.tile([1, 128], fp16)
        nc.vector.memset(ones[:], 1.0)
        zero = const.tile([128, 1], fp16)
        nc.vector.memset(zero[:], 0.0)

        # ---- drive terms + scans + linear1, interleaved per j = dir*4 + m ----
        # a = WfeB @ onehot in 8 PSUM blocks of 16 chains; ScalarE lays each
        # block into the scan layout [p, b*33 + s]; the DVE scan computes
        # h = relu(a + h_prev) for all 128 chains in one instruction; then
        # the two W1 groups for this j stream in and accumulate into psum_h1.
        # W1 group order is (dir, m)-major so group G only needs scan j = G//8.
        psum_h1 = ps_h1.tile([128, 512], fp32)
        bias_done = [False]

        def a_phase(j):
            dd, m = j // 4, j % 4
            a_sb = a_pool.tile([128, FREE], fp16, tag="a")
            sep = a_sb[:].rearrange("p (b t) -> p b t", t=CL)[:, :, T]
            nc.vector.memset(sep, NEG)
            lhsT = wfe[:, dd * HID + m * 128: dd * HID + m * 128 + 128]
            for q in range(8):
                pa = ps_a.tile([128, 512], fp32, tag="pa")
                nc.tensor.matmul(
                    pa[:], lhsT, ohsb[dd][:, q * 512:(q + 1) * 512],
                    start=True, stop=True,
                )
                av = a_sb[:].rearrange("p (b t) -> p b t", t=CL)[:, 16 * q:16 * q + 16, 0:T]
                pv = pa[:].rearrange("p (b t) -> p b t", t=T)
                nc.scalar.copy(av, pv)
            h_t = h_pool.tile([128, FREE], fp16, tag="h")
            if j == 0:
                # head-critical: scan in q-pair quarters, chasing the copies
                # (q<4 copies split Act/DVE; q>=4 whole on ScalarE so
                # VectorE can scan continuously)
                QC = 32 * CL
                for i in range(4):
                    nc.vector.tensor_tensor_scan(
                        h_t[:, i * QC:(i + 1) * QC], a_sb[:, i * QC:(i + 1) * QC],
                        zero[:].broadcast_to([128, QC]),
                        initial=0.0, op0=ALU.add, op1=ALU.max,
                    )
            else:
                nc.vector.tensor_tensor_scan(
                    h_t[:], a_sb[:], zero[:].broadcast_to([128, FREE]),
                    initial=0.0, op0=ALU.add, op1=ALU.max,
                )
            # e4m3 copy of the t<4 slice: both DoubleRow operands must be
            # fp8e4; the early hidden states are small so the quantization
            # noise is cheap (err ~ h_t * dW)
            x8 = x8_pool.tile([128, 512], fp8e4, tag="x8")
            nc.scalar.copy(
                x8[:].rearrange("p (t b) -> p t b", t=4),
                h_t[:].rearrange("p (b t) -> p t b", t=CL)[:, 0:4, :],
            )
            return h_t, x8

        # per-j group order: big fp16 groups first so the DMA stream runs
        # ahead of PE consumption (the fp8 groups at the end need half the
        # bytes per PE-cycle); tg0 (DoubleRow) at slot 2 so its e4m3
        # operand copy hides behind the first two groups
        TG_ORDER = (5, 6, 1, 7, 2, 3, 4, 0)
        hs = {0: a_phase(0), 1: a_phase(1), 2: a_phase(2)}
        for j in range(8):
            h_t, x8 = hs[j]
            hv = h_t[:].rearrange("p (b t) -> p t b", t=CL)
            for gi, tg in enumerate(TG_ORDER):
                flat = j * 8 + gi
                if tg == 0:
                    src_ap, wdt = d["W1D"][j], fp8e4
                elif tg <= 4:
                    src_ap, wdt = d["W1A"][j * 4 + tg - 1], fp8
                else:
                    src_ap, wdt = d["W1B"][j * 3 + tg - 5], fp16
                w_t = w1_pre.pop(flat, None)
                if w_t is None:
                    w_t = w1_pool.tile([128, 2048], wdt, tag="w_t")
                    nc.sync.dma_start(w_t[:], src_ap)
                last_grp = flat == W1_GRP - 1
                if not bias_done[0]:
                    # rank-1 bias opens the accumulation: ones.T @ b1
                    # broadcasts b1 over the batch partitions
                    nc.tensor.matmul(psum_h1[:], ones[:], b1sb,
                                     start=True, stop=False)
                    bias_done[0] = True
                if tg == 0:
                    # DoubleRow: contracts two k-tiles per instruction at
                    # 0.5 cycles/col; group layout [p, pair, slot, n]
                    wv = w_t[:].rearrange("p (pr sl n) -> p pr sl n", pr=2, sl=2)
                    x8v = x8[:].rearrange("p (t b) -> p t b", t=4)
                    for pair in range(2):
                        for nh in range(2):
                            nc.tensor.matmul(
                                psum_h1[:, nh * 256:(nh + 1) * 256],
                                x8v[:, 2 * pair:2 * pair + 2, :],
                                wv[:, pair, :, nh * 256:(nh + 1) * 256],
                                start=False,
                                stop=(last_grp and pair == 1),
                                perf_mode=mybir.MatmulPerfMode.DoubleRow,
                            )
                else:
                    for c in range(4):
                        t_idx = tg * 4 + c
                        nc.tensor.matmul(
                            psum_h1[:], hv[:, t_idx, :], w_t[:, c * 512:(c + 1) * 512],
                            start=False, stop=(last_grp and c == 3),
                        )
                if gi == 0 and j + 3 < 8:
                    if j == 1:
                        # dir-1 onehot: first needed by a_phase(4), emitted
                        # at the top of j=1; keeps it off the head path
                        nc.sync.dma_start(ohsb[1], d["OHX"][:, BL * T:2 * BL * T])
                    hs[j + 3] = a_phase(j + 3)
        # tail-only weights ship after the W1 stream so the last W1 byte
        # (the critical one) arrives ~1.7us earlier; this DMA overlaps the
        # h1 drain + transposes and lands before the first layer matmul
        nc.sync.dma_start(w2o[:], d["W2O"][:])
        a_ctx.close()  # release the a-phase PSUM banks for the tail pools
        h1sb = const.tile([128, 512], fp16)
        nc.scalar.activation(h1sb[:], psum_h1[:], AF.Relu)
        h1_ctx.close()
        ps_t = ctx.enter_context(tc.tile_pool(name="pst", bufs=1, space="PSUM"))
        ps_l = ctx.enter_context(tc.tile_pool(name="psl", bufs=4, space="PSUM"))
        ps_o = ctx.enter_context(tc.tile_pool(name="pso", bufs=2, space="PSUM"))

        # ---- transpose h1 to feature-major [512, 128] ----
        # Twin PSUM banks: ScalarE drains one while VectorE drains the other
        # (Tile serializes same-bank readers)
        pt_a = ps_t.tile([128, 256], fp16, tag="pta")
        pt_b = ps_t.tile([128, 256], fp16, tag="ptb")
        cur = hp_pool.tile([128, 512], fp16, tag="hp")
        for m in (0, 1):
            nc.tensor.transpose(
                pt_a[:, (m % 2) * 128:(m % 2) * 128 + 128],
                h1sb[:, m * 128:(m + 1) * 128], idsb[:])
        nc.scalar.copy(cur[:, 0:256], pt_a[:])
        for m in (2, 3):
            nc.tensor.transpose(
                pt_b[:, (m % 2) * 128:(m % 2) * 128 + 128],
                h1sb[:, m * 128:(m + 1) * 128], idsb[:])
        nc.vector.tensor_copy(cur[:, 256:512], pt_b[:])

        # ---- 4 x (h = relu(W2 @ h' + b2)), feature-major, col block = m ----
        # per-m psum tiles so each 128-col block drains (Act/DVE alternating)
        # as soon as its 4 k-matmuls finish, and the next layer's k-matmuls
        # chase the drains instead of waiting for the full 512
        for _L in range(4):
            hq = hp_pool.tile([128, 512], fp16, tag="hp")
            for m in range(4):
                pl = ps_l.tile([128, 128], fp32, tag="pl")
                nc.tensor.matmul(
                    pl[:], b2r[:, m * 128:(m + 1) * 128], ones[:],
                    start=True, stop=False,
                )
                for k in range(4):
                    nc.tensor.matmul(
                        pl[:],
                        w2sb[:, k * 512 + m * 128: k * 512 + m * 128 + 128],
                        cur[:, k * 128:(k + 1) * 128],
                        start=False, stop=(k == 3),
                    )
                hsl = hq[:, m * 128:(m + 1) * 128]
                if m % 2 == 0:
                    nc.scalar.activation(hsl, pl[:], AF.Relu)
                else:
                    nc.vector.tensor_scalar_max(hsl, pl[:], 0.0)
            cur = hq

        # ---- output head: out' = Wo @ h' + bo  -> [97, 128] ----
        # batch halves on separate banks so the two drains run in parallel
        osb = const.tile([MOD, BL], fp32)
        for hh in range(2):
            po = ps_o.tile([MOD, 64], fp32, tag="po")
            nc.tensor.matmul(po[:], bor, ones[:, 0:64], start=True, stop=False)
            for k in range(4):
                nc.tensor.matmul(
                    po[:], wosb[:, k * MOD:(k + 1) * MOD],
                    cur[:, k * 128 + hh * 64: k * 128 + hh * 64 + 64],
                    start=False, stop=(k == 3),
                )
            if hh == 0:
                nc.scalar.copy(osb[:, 0:64], po[:])
            else:
                nc.vector.tensor_copy(osb[:, 64:128], po[:])
        nc.sync.dma_start(d["OUT"], osb[:])


def _host_prep(inputs):
    x = np.asarray(inputs["x"]).astype(np.int64)          # [B, T]
    emb = np.asarray(inputs["emb"], np.float32)           # [97, 512]
    Wf = np.asarray(inputs["Wf"], np.float32)
    bf = np.asarray(inputs["bf"], np.float32)
    Wb = np.asarray(inputs["Wb"], np.float32)
    bb = np.asarray(inputs["bb"], np.float32)
    W1 = np.asarray(inputs["W1"], np.float32)             # [512, 32768]
    b1 = np.asarray(inputs["b1"], np.float32)
    W2 = np.asarray(inputs["W2"], np.float32)
    b2 = np.asarray(inputs["b2"], np.float32)
    Wo = np.asarray(inputs["Wo"], np.float32)             # [97, 512]
    bo = np.asarray(inputs["bo"], np.float32)

    # fold embedding gather + input projection + bias:
    # a_d[:, b, s] = (Wd @ emb.T + bd)[:, idx] since onehot has exactly one 1
    WFE = np.ascontiguousarray(np.stack([
        (Wf @ emb.T + bf[:, None]).T,                     # [97, 512]
        (Wb @ emb.T + bb[:, None]).T,
    ]).transpose(1, 0, 2).reshape(MOD, 2 * HID)).astype(F16)

    # per-core one-hot of x, col = b*32 + s; fwd s = t, bwd s = reversed t.
    # Pure reformatting of the index tensor; 0/1 are exact in fp8e3.
    xc = x.reshape(NCORES, BL, T)
    XR = np.concatenate([
        xc.reshape(NCORES, BL * T), xc[:, :, ::-1].reshape(NCORES, BL * T)
    ], axis=1)                                            # [NC, 8192] int
    OHX = (XR[:, None, :] == np.arange(MOD)[None, :, None]).astype(E3M4)
    IDA = np.concatenate([
        np.eye(128, dtype=np.float32),
        np.arange(128, dtype=np.float32).reshape(128, 1),
    ], axis=1).astype(F16)

    # W1 -> [64, 128, 2048]: group G = (d, m, tg) holds k-chunks for
    # t = 4*tg .. 4*tg+3 of direction d, hid-tile m, side by side
    # W1.T row layout is [t, d, m, p]-major (xcat col = t*1024 + d*512 + m*128)
    # Everything is pre-scaled by SW; tg < 4 (t < 16, where the RNN states are
    # still small) ships as fp8e3, the rest as fp16. The psum is SW-scaled,
    # undone by shipping Wo/SW.
    W1S = (
        W1.T.reshape(8, 4, 2, 4, 128, 512)       # [tg, tc, d, m, p, col]
        .transpose(2, 3, 0, 4, 1, 5)             # [d, m, tg, p, tc, col]
        .reshape(W1_GRP, 128, 2048)
    ) * SW
    tgm = np.arange(W1_GRP) % 8
    W1D = np.ascontiguousarray(W1S[tgm == 0]).astype(F8E4)            # [8, ...] DoubleRow
    W1A = np.ascontiguousarray(W1S[(tgm >= 1) & (tgm <= 4)]).astype(E3M4)  # [32, ...]
    W1B = np.ascontiguousarray(W1S[tgm >= 5]).astype(F16)             # [24, ...]
    W2S = np.ascontiguousarray(W2.T.reshape(4, 128, 512).transpose(1, 0, 2).reshape(128, 2048)).astype(F16)
    WOS = np.ascontiguousarray((Wo.T / SW).reshape(4, 128, MOD).transpose(1, 0, 2).reshape(128, 4 * MOD)).astype(F16)
    W2O = np.concatenate([W2S, WOS], axis=1)
    BIAH = np.concatenate([b1 * SW, b2 * SW, bo]).astype(F16).reshape(1, -1)  # [1, 1121]

    shared = {"WFE": WFE, "W1D": W1D, "W1A": W1A, "W1B": W1B, "W2O": W2O, "IDA": IDA, "BIA": BIAH}
    in_maps = [dict(shared, OHX=OHX[c]) for c in range(NCORES)]
    return in_maps


def _get_nc():
    if "nc" not in _CACHE:
        _CACHE["nc"] = _build()
    return _CACHE["nc"]


def kernel(**inputs):
    from concourse.bass_utils import run_bass_kernel_spmd

    nc = _get_nc()
    in_maps = _host_prep(inputs)
    res = run_bass_kernel_spmd(nc, in_maps, list(range(NCORES)))
    outs = [np.asarray(res.results[c]["OUT"], np.float32) for c in range(NCORES)]
    return np.ascontiguousarray(np.concatenate([o.T for o in outs], axis=0))  # [1024, 97]



# revision 43
# speedup vs baseline: 1.0106x; 1.0106x over previous
"""Trainium2 Bass kernel for the BiDirectionalRNN problem.

Math (matches the fp32 jax reference):
    e = emb[x]                                   # [B, T, 512]
    fwd:  h_t = relu(e_t @ Wf.T + bf + h_{t-1})  # fs[t]
    bwd over reversed e: bs[s]                   # generation order
    xcat = concat_t [fs[t], bs[t]]  -> [B, T*1024]
    h1 = relu(xcat @ W1.T + b1); 4x h = relu(h @ W2.T + b2); out = h @ Wo.T + bo

Strategy:
  * Data-parallel over batch: 1024/8 = 128 samples per NeuronCore.
  * Host folds embedding + input projection weights into per-direction
    tables WfeB = Wf @ emb.T + bf ([97, 512] per dir, fp16) and builds
    the one-hot of x ([97, 2*BL*T] fp8e3 — pure index reformatting). The
    drive terms a = WfeB @ onehot are K=97 matmuls (fp16 x fp8 mixed).
  * ScalarE copies each a-GEMM PSUM block into the scan layout
    [p, b*33 + s] (strided 3D AP), separator column = -60000 (fp16).
  * The 32-step recurrence h = relu(a + h_prev) runs as a DVE
    tensor_tensor_scan per (dir, hid-tile): state=(a add state) max 0,
    fp32 internal state; the separator resets state to 0 between chains.
    The first scan (the pipeline head) is split into q-pair quarters that
    chase the ScalarE PSUM copies, so the copy chain and the scan chain
    pipeline across the two engines.
  * W1 ships pre-scaled by SW=256 in three precisions, keyed to how the
    RNN state magnitude (and hence the error weight h_t*dW) grows with t:
    t<4 as fp8 e4m3 run in DoubleRow perf mode (both operands fp8e4,
    2 k-tiles per instruction at 0.5 cycles/col — a per-j e4m3 copy of
    the t<4 scan slice feeds the stationary side), t in [4,20) as fp8
    e3m4 (moving-operand only: matmul speed keys off the MOVING dtype
    and mixed input dtypes are allowed, so the xcat side stays fp16),
    t>=20 as fp16. 22MB/core instead of 32MB bf16; rel err 1.66e-2 vs
    the 2e-2 gate. Groups are (dir, m)-major, within each j ordered
    fp16-first / DoubleRow-last so the DMA stream runs ahead of PE and
    the end-of-stream critical chain is the smallest group. The GEMM
    accumulates into one PSUM bank; lhsT = strided scan-output views;
    the SW scale rides through the relu chain (relu(s*x) = s*relu(x))
    and is undone by shipping Wo/SW. a-phases pipeline three js ahead.
  * Tail: PE-transpose h1, then 4 x [512,512] + [97,512] in transposed
    (feature-major) layout; per-m [128,128] PSUM tiles drain alternately
    on ScalarE/VectorE so the next layer's k-matmuls chase the drains;
    the output head runs in batch halves on twin PSUM banks. Biases enter
    PSUM via rank-1 matmuls (b1/b2 pre-scaled by SW on host).
  * DMA order follows the critical path: dir-0 WFE + onehot first (the
    first a-matmul gates everything), W1 stream next, dir-1 onehot inside
    the loop at j=1, tail-only W2/Wo after the W1 stream. a-phase PSUM is
    a 7-deep ring + 1 bank for the W1 accumulator; the a-ring is released
    after the GEMM so the tail pools (2+4+2 banks) fit in the 8 banks.
"""

import numpy as np
import ml_dtypes

F16 = np.float16
E3M4 = ml_dtypes.float8_e3m4
F8E4 = ml_dtypes.float8_e4m3
SW = 256.0            # W1 pre-scale; psum is SW-scaled, undone in the head
NT8 = 20              # t < NT8 ships as fp8e3 (RNN states are small early on)

MOD = 97
HID = 512
T = 32
B = 1024
NCORES = 8
BL = B // NCORES          # 128 batch per core
CL = T + 1                # chain length incl. separator column
FREE = BL * CL            # 4224 scan columns per tile
NEG = -60000.0
W1_GRP = 64               # W1 DMA groups of 4 k-chunks (512KB each)

_CACHE: dict = {}


def _build(nobias=False, reps=1):
    import concourse.tile as tile
    from concourse import bacc, mybir

    fp32 = mybir.dt.float32
    fp16 = mybir.dt.float16
    fp8 = mybir.dt.float8e3
    fp8e4 = mybir.dt.float8e4

    nc = bacc.Bacc(
        "TRN2", target_bir_lowering=False, debug=False, num_devices=NCORES
    )

    d = {
        "IDA": nc.dram_tensor("IDA", [128, 129], fp16, kind="ExternalInput").ap(),
        "WFE": nc.dram_tensor("WFE", [MOD, 2 * HID], fp16, kind="ExternalInput").ap(),
        "W1D": nc.dram_tensor("W1D", [8, 128, 2048], fp8e4, kind="ExternalInput").ap(),
        "W1A": nc.dram_tensor("W1A", [W1_GRP // 2, 128, 2048], fp8, kind="ExternalInput").ap(),
        "W1B": nc.dram_tensor("W1B", [W1_GRP * 3 // 8, 128, 2048], fp16, kind="ExternalInput").ap(),
        "W2O": nc.dram_tensor("W2O", [128, 4 * 512 + 4 * MOD], fp16, kind="ExternalInput").ap(),
        "BIA": nc.dram_tensor("BIA", [1, 1121], fp16, kind="ExternalInput").ap(),
        "OHX": nc.dram_tensor("OHX", [MOD, 2 * BL * T], fp8, kind="ExternalInput").ap(),
        "OUT": nc.dram_tensor("OUT", [MOD, BL], fp32, kind="ExternalOutput").ap(),
    }

    with tile.TileContext(nc) as tc:
        for _ in range(reps):
            _emit(tc, d, mybir, nobias)

    nc.compile()
    return nc


def _emit(tc, d, mybir, nobias=False):
    nc = tc.nc
    fp32 = mybir.dt.float32
    fp16 = mybir.dt.float16
    fp8 = mybir.dt.float8e3
    fp8e4 = mybir.dt.float8e4
    AF = mybir.ActivationFunctionType
    ALU = mybir.AluOpType

    from contextlib import ExitStack

    with ExitStack() as ctx:
        const = ctx.enter_context(tc.tile_pool(name="const", bufs=1))
        a_pool = ctx.enter_context(tc.tile_pool(name="apool", bufs=3))
        h_pool = ctx.enter_context(tc.tile_pool(name="hpool", bufs=4))
        w1_pool = ctx.enter_context(tc.tile_pool(name="w1pool", bufs=24))
        hp_pool = ctx.enter_context(tc.tile_pool(name="hppool", bufs=3))
        x8_pool = ctx.enter_context(tc.tile_pool(name="x8pool", bufs=4))
        h1_ctx = ExitStack()
        ps_h1 = h1_ctx.enter_context(tc.tile_pool(name="psh1", bufs=1, space="PSUM"))
        a_ctx = ExitStack()
        ps_a = a_ctx.enter_context(tc.tile_pool(name="psa", bufs=7, space="PSUM"))

        # ---- head ----
        # HWDGE descriptor generations serialize (~0.65us each), so DMA
        # instruction order IS the critical path: dir-0 WFE + onehot feed
        # the first a-phase, then the W1 stream; dir-1 onehot ships inside
        # the loop (first needed at j=4)
        ohall = const.tile([MOD, 2 * BL * T], fp8)
        ohsb = [ohall[:, 0:BL * T], ohall[:, BL * T:2 * BL * T]]
        wfe = const.tile([MOD, 2 * HID], fp16)
        nc.sync.dma_start(wfe[:, 0:HID], d["WFE"][:, 0:HID])
        nc.sync.dma_start(ohsb[0][:, 0:1024], d["OHX"][:, 0:1024])
        nc.sync.dma_start(ohsb[0][:, 1024:4096], d["OHX"][:, 1024:4096])
        if not nobias:
            bia = const.tile([1, 1121], fp16)
            nc.sync.dma_start(bia[:], d["BIA"])
            b1sb = bia[:, 0:512]
            b2r = bia[:, 512:1024]
            bor = bia[:, 1024:1121]
        nc.sync.dma_start(wfe[:, HID:2 * HID], d["WFE"][:, HID:2 * HID])
        w1_pre = {}
        for G in (0, 1):
            w_t = w1_pool.tile([128, 2048], fp16, tag="w_t")
            nc.sync.dma_start(w_t[:], d["W1B"][G])
            w1_pre[G] = w_t
        ida = const.tile([128, 129], fp16)
        nc.sync.dma_start(ida[:], d["IDA"])
        idsb = ida[:, 0:128]
        w2o = const.tile([128, 4 * 512 + 4 * MOD], fp16)
        w2sb = w2o[:, 0:2048]
        wosb = w2o[:, 2048:2048 + 4 * MOD]
        ones = const.tile([1, 128], fp16)
        nc.vector.memset(ones[:], 1.0)
        zero = const.tile([128, 1], fp16)
        nc.vector.memset(zero[:], 0.0)

        # ---- drive terms + scans + linear1, interleaved per j = dir*4 + m ----
        # a = WfeB @ onehot in 8 PSUM blocks of 16 chains; ScalarE lays each
        # block into the scan layout [p, b*33 + s]; the DVE scan computes
        # h = relu(a + h_prev) for all 128 chains in one instruction; then
        # the two W1 groups for this j stream in and accumulate into psum_h1.
        # W1 group order is (dir, m)-major so group G only needs scan j = G//8.
        psum_h1 = ps_h1.tile([128, 512], fp32)
        bias_done = [False]

        def a_phase(j):
            dd, m = j // 4, j % 4
            a_sb = a_pool.tile([128, FREE], fp16, tag="a")
            sep = a_sb[:].rearrange("p (b t) -> p b t", t=CL)[:, :, T]
            nc.vector.memset(sep, NEG)
            lhsT = wfe[:, dd * HID + m * 128: dd * HID + m * 128 + 128]
            for q in range(8):
                pa = ps_a.tile([128, 512], fp32, tag="pa")
                nc.tensor.matmul(
                    pa[:], lhsT, ohsb[dd][:, q * 512:(q + 1) * 512],
                    start=True, stop=True,
                )
                av = a_sb[:].rearrange("p (b t) -> p b t", t=CL)[:, 16 * q:16 * q + 16, 0:T]
                pv = pa[:].rearrange("p (b t) -> p b t", t=T)
                nc.scalar.copy(av, pv)
            h_t = h_pool.tile([128, FREE], fp16, tag="h")
            if j == 0:
                # head-critical: scan in q-pair quarters, chasing the copies
                # (q<4 copies split Act/DVE; q>=4 whole on ScalarE so
                # VectorE can scan continuously)
                QC = 32 * CL
                for i in range(4):
                    nc.vector.tensor_tensor_scan(
                        h_t[:, i * QC:(i + 1) * QC], a_sb[:, i * QC:(i + 1) * QC],
                        zero[:].broadcast_to([128, QC]),
                        initial=0.0, op0=ALU.add, op1=ALU.max,
                    )
            else:
                nc.vector.tensor_tensor_scan(
                    h_t[:], a_sb[:], zero[:].broadcast_to([128, FREE]),
                    initial=0.0, op0=ALU.add, op1=ALU.max,
                )
            # e4m3 copy of the t<4 slice: both DoubleRow operands must be
            # fp8e4; the early hidden states are small so the quantization
            # noise is cheap (err ~ h_t * dW)
            x8 = x8_pool.tile([128, 512], fp8e4, tag="x8")
            nc.scalar.copy(
                x8[:].rearrange("p (t b) -> p t b", t=4),
                h_t[:].rearrange("p (b t) -> p t b", t=CL)[:, 0:4, :],
            )
            return h_t, x8

        # per-j group order: big fp16 groups first so the DMA stream runs
        # ahead of PE consumption (the fp8 groups at the end need half the
        # bytes per PE-cycle); tg0 (DoubleRow) at slot 2 so its e4m3
        # operand copy hides behind the first two groups
        TG_ORDER = (5, 6, 1, 7, 2, 3, 4, 0)
        hs = {0: a_phase(0), 1: a_phase(1), 2: a_phase(2)}
        for j in range(8):
            h_t, x8 = hs[j]
            hv = h_t[:].rearrange("p (b t) -> p t b", t=CL)
            for gi, tg in enumerate(TG_ORDER):
                flat = j * 8 + gi
                if tg == 0:
                    src_ap, wdt = d["W1D"][j], fp8e4
                elif tg <= 4:
                    src_ap, wdt = d["W1A"][j * 4 + tg - 1], fp8
                else:
                    src_ap, wdt = d["W1B"][j * 3 + tg - 5], fp16
                w_t = w1_pre.pop(flat, None)
                if w_t is None:
                    w_t = w1_pool.tile([128, 2048], wdt, tag="w_t")
                    nc.sync.dma_start(w_t[:], src_ap)
                last_grp = flat == W1_GRP - 1
                opener = False
                if not bias_done[0]:
                    if nobias:
                        # all biases are zero (checked at runtime in
                        # kernel()): the first W1 matmul opens the group
                        opener = True
                    else:
                        # rank-1 bias opens the accumulation: ones.T @ b1
                        # broadcasts b1 over the batch partitions
                        nc.tensor.matmul(psum_h1[:], ones[:], b1sb,
                                         start=True, stop=False)
                    bias_done[0] = True
                if tg == 0:
                    # DoubleRow: contracts two k-tiles per instruction at
                    # 0.5 cycles/col; group layout [p, pair, slot, n]
                    wv = w_t[:].rearrange("p (pr sl n) -> p pr sl n", pr=2, sl=2)
                    x8v = x8[:].rearrange("p (t b) -> p t b", t=4)
                    for pair in range(2):
                        for nh in range(2):
                            nc.tensor.matmul(
                                psum_h1[:, nh * 256:(nh + 1) * 256],
                                x8v[:, 2 * pair:2 * pair + 2, :],
                                wv[:, pair, :, nh * 256:(nh + 1) * 256],
                                start=False,
                                stop=(last_grp and pair == 1),
                                perf_mode=mybir.MatmulPerfMode.DoubleRow,
                            )
                else:
                    for c in range(4):
                        t_idx = tg * 4 + c
                        nc.tensor.matmul(
                            psum_h1[:], hv[:, t_idx, :], w_t[:, c * 512:(c + 1) * 512],
                            start=(opener and c == 0), stop=(last_grp and c == 3),
                        )
                if gi == 0 and j + 3 < 8:
                    if j == 1:
                        # dir-1 onehot: first needed by a_phase(4), emitted
                        # at the top of j=1; keeps it off the head path
                        nc.sync.dma_start(ohsb[1], d["OHX"][:, BL * T:2 * BL * T])
                    hs[j + 3] = a_phase(j + 3)
        # tail-only weights ship after the W1 stream so the last W1 byte
        # (the critical one) arrives ~1.7us earlier; this DMA overlaps the
        # h1 drain + transposes and lands before the first layer matmul
        nc.sync.dma_start(w2o[:], d["W2O"][:])
        a_ctx.close()  # release the a-phase PSUM banks for the tail pools
        h1sb = const.tile([128, 512], fp16)
        nc.scalar.activation(h1sb[:], psum_h1[:], AF.Relu)
        h1_ctx.close()
        ps_t = ctx.enter_context(tc.tile_pool(name="pst", bufs=1, space="PSUM"))
        ps_l = ctx.enter_context(tc.tile_pool(name="psl", bufs=4, space="PSUM"))
        ps_o = ctx.enter_context(tc.tile_pool(name="pso", bufs=2, space="PSUM"))

        # ---- transpose h1 to feature-major [512, 128] ----
        # Twin PSUM banks: ScalarE drains one while VectorE drains the other
        # (Tile serializes same-bank readers)
        pt_a = ps_t.tile([128, 256], fp16, tag="pta")
        pt_b = ps_t.tile([128, 256], fp16, tag="ptb")
        cur = hp_pool.tile([128, 512], fp16, tag="hp")
        for m in (0, 1):
            nc.tensor.transpose(
                pt_a[:, (m % 2) * 128:(m % 2) * 128 + 128],
                h1sb[:, m * 128:(m + 1) * 128], idsb[:])
        nc.scalar.copy(cur[:, 0:256], pt_a[:])
        for m in (2, 3):
            nc.tensor.transpose(
                pt_b[:, (m % 2) * 128:(m % 2) * 128 + 128],
                h1sb[:, m * 128:(m + 1) * 128], idsb[:])
        nc.vector.tensor_copy(cur[:, 256:512], pt_b[:])

        # ---- 4 x (h = relu(W2 @ h' + b2)), feature-major, col block = m ----
        # per-m psum tiles so each 128-col block drains (Act/DVE alternating)
        # as soon as its 4 k-matmuls finish, and the next layer's k-matmuls
        # chase the drains instead of waiting for the full 512
        for _L in range(4):
            hq = hp_pool.tile([128, 512], fp16, tag="hp")
            for m in range(4):
                pl = ps_l.tile([128, 128], fp32, tag="pl")
                if not nobias:
                    nc.tensor.matmul(
                        pl[:], b2r[:, m * 128:(m + 1) * 128], ones[:],
                        start=True, stop=False,
                    )
                for k in range(4):
                    nc.tensor.matmul(
                        pl[:],
                        w2sb[:, k * 512 + m * 128: k * 512 + m * 128 + 128],
                        cur[:, k * 128:(k + 1) * 128],
                        start=(nobias and k == 0), stop=(k == 3),
                    )
                hsl = hq[:, m * 128:(m + 1) * 128]
                if m % 2 == 0:
                    nc.scalar.activation(hsl, pl[:], AF.Relu)
                else:
                    nc.vector.tensor_scalar_max(hsl, pl[:], 0.0)
            cur = hq

        # ---- output head: out' = Wo @ h' + bo  -> [97, 128] ----
        # batch halves on separate banks so the two drains run in parallel
        osb = const.tile([MOD, BL], fp32)
        for hh in range(2):
            po = ps_o.tile([MOD, 64], fp32, tag="po")
            if not nobias:
                nc.tensor.matmul(po[:], bor, ones[:, 0:64], start=True, stop=False)
            for k in range(4):
                nc.tensor.matmul(
                    po[:], wosb[:, k * MOD:(k + 1) * MOD],
                    cur[:, k * 128 + hh * 64: k * 128 + hh * 64 + 64],
                    start=(nobias and k == 0), stop=(k == 3),
                )
            if hh == 0:
                nc.scalar.copy(osb[:, 0:64], po[:])
            else:
                nc.vector.tensor_copy(osb[:, 64:128], po[:])
        nc.sync.dma_start(d["OUT"], osb[:])


def _host_prep(inputs):
    x = np.asarray(inputs["x"]).astype(np.int64)          # [B, T]
    emb = np.asarray(inputs["emb"], np.float32)           # [97, 512]
    Wf = np.asarray(inputs["Wf"], np.float32)
    bf = np.asarray(inputs["bf"], np.float32)
    Wb = np.asarray(inputs["Wb"], np.float32)
    bb = np.asarray(inputs["bb"], np.float32)
    W1 = np.asarray(inputs["W1"], np.float32)             # [512, 32768]
    b1 = np.asarray(inputs["b1"], np.float32)
    W2 = np.asarray(inputs["W2"], np.float32)
    b2 = np.asarray(inputs["b2"], np.float32)
    Wo = np.asarray(inputs["Wo"], np.float32)             # [97, 512]
    bo = np.asarray(inputs["bo"], np.float32)

    # fold embedding gather + input projection + bias:
    # a_d[:, b, s] = (Wd @ emb.T + bd)[:, idx] since onehot has exactly one 1
    WFE = np.ascontiguousarray(np.stack([
        (Wf @ emb.T + bf[:, None]).T,                     # [97, 512]
        (Wb @ emb.T + bb[:, None]).T,
    ]).transpose(1, 0, 2).reshape(MOD, 2 * HID)).astype(F16)

    # per-core one-hot of x, col = b*32 + s; fwd s = t, bwd s = reversed t.
    # Pure reformatting of the index tensor; 0/1 are exact in fp8e3.
    xc = x.reshape(NCORES, BL, T)
    XR = np.concatenate([
        xc.reshape(NCORES, BL * T), xc[:, :, ::-1].reshape(NCORES, BL * T)
    ], axis=1)                                            # [NC, 8192] int
    OHX = (XR[:, None, :] == np.arange(MOD)[None, :, None]).astype(E3M4)
    IDA = np.concatenate([
        np.eye(128, dtype=np.float32),
        np.arange(128, dtype=np.float32).reshape(128, 1),
    ], axis=1).astype(F16)

    # W1 -> [64, 128, 2048]: group G = (d, m, tg) holds k-chunks for
    # t = 4*tg .. 4*tg+3 of direction d, hid-tile m, side by side
    # W1.T row layout is [t, d, m, p]-major (xcat col = t*1024 + d*512 + m*128)
    # Everything is pre-scaled by SW; tg < 4 (t < 16, where the RNN states are
    # still small) ships as fp8e3, the rest as fp16. The psum is SW-scaled,
    # undone by shipping Wo/SW.
    W1S = (
        W1.T.reshape(8, 4, 2, 4, 128, 512)       # [tg, tc, d, m, p, col]
        .transpose(2, 3, 0, 4, 1, 5)             # [d, m, tg, p, tc, col]
        .reshape(W1_GRP, 128, 2048)
    ) * SW
    tgm = np.arange(W1_GRP) % 8
    W1D = np.ascontiguousarray(W1S[tgm == 0]).astype(F8E4)            # [8, ...] DoubleRow
    W1A = np.ascontiguousarray(W1S[(tgm >= 1) & (tgm <= 4)]).astype(E3M4)  # [32, ...]
    W1B = np.ascontiguousarray(W1S[tgm >= 5]).astype(F16)             # [24, ...]
    W2S = np.ascontiguousarray(W2.T.reshape(4, 128, 512).transpose(1, 0, 2).reshape(128, 2048)).astype(F16)
    WOS = np.ascontiguousarray((Wo.T / SW).reshape(4, 128, MOD).transpose(1, 0, 2).reshape(128, 4 * MOD)).astype(F16)
    W2O = np.concatenate([W2S, WOS], axis=1)
    BIAH = np.concatenate([b1 * SW, b2 * SW, bo]).astype(F16).reshape(1, -1)  # [1, 1121]

    shared = {"WFE": WFE, "W1D": W1D, "W1A": W1A, "W1B": W1B, "W2O": W2O, "IDA": IDA, "BIA": BIAH}
    in_maps = [dict(shared, OHX=OHX[c]) for c in range(NCORES)]
    return in_maps


def _get_nc(nobias=True):
    key = ("nc", nobias)
    if key not in _CACHE:
        _CACHE[key] = _build(nobias)
    return _CACHE[key]


def kernel(**inputs):
    from concourse.bass_utils import run_bass_kernel_spmd

    nobias = all(
        not np.any(np.asarray(inputs[k], np.float32))
        for k in ("bf", "bb", "b1", "b2", "bo")
    )
    nc = _get_nc(nobias)
    in_maps = _host_prep(inputs)
    res = run_bass_kernel_spmd(nc, in_maps, list(range(NCORES)))
    outs = [np.asarray(res.results[c]["OUT"], np.float32) for c in range(NCORES)]
    return np.ascontiguousarray(np.concatenate([o.T for o in outs], axis=0))  # [1024, 97]



# revision 45
# speedup vs baseline: 1.0130x; 1.0024x over previous
"""Trainium2 Bass kernel for the BiDirectionalRNN problem.

Math (matches the fp32 jax reference):
    e = emb[x]                                   # [B, T, 512]
    fwd:  h_t = relu(e_t @ Wf.T + bf + h_{t-1})  # fs[t]
    bwd over reversed e: bs[s]                   # generation order
    xcat = concat_t [fs[t], bs[t]]  -> [B, T*1024]
    h1 = relu(xcat @ W1.T + b1); 4x h = relu(h @ W2.T + b2); out = h @ Wo.T + bo

Strategy:
  * Data-parallel over batch: 1024/8 = 128 samples per NeuronCore.
  * Host folds embedding + input projection weights into per-direction
    tables WfeB = Wf @ emb.T + bf ([97, 512] per dir, fp16) and builds
    the one-hot of x ([97, 2*BL*T] fp8e3 — pure index reformatting). The
    drive terms a = WfeB @ onehot are K=97 matmuls (fp16 x fp8 mixed).
  * ScalarE copies each a-GEMM PSUM block into the scan layout
    [p, b*33 + s] (strided 3D AP), separator column = -60000 (fp16).
  * The 32-step recurrence h = relu(a + h_prev) runs as a DVE
    tensor_tensor_scan per (dir, hid-tile): state=(a add state) max 0,
    fp32 internal state; the separator resets state to 0 between chains.
    The first scan (the pipeline head) is split into q-pair quarters that
    chase the ScalarE PSUM copies, so the copy chain and the scan chain
    pipeline across the two engines.
  * W1 ships pre-scaled by SW=256 in three precisions, keyed to how the
    RNN state magnitude (and hence the error weight h_t*dW) grows with t:
    t<4 as fp8 e4m3 run in DoubleRow perf mode (both operands fp8e4,
    2 k-tiles per instruction at 0.5 cycles/col — a per-j e4m3 copy of
    the t<4 scan slice feeds the stationary side), t in [4,20) as fp8
    e3m4 (moving-operand only: matmul speed keys off the MOVING dtype
    and mixed input dtypes are allowed, so the xcat side stays fp16),
    t>=20 as fp16. 22MB/core instead of 32MB bf16; rel err 1.66e-2 vs
    the 2e-2 gate. Groups are (dir, m)-major, within each j ordered
    fp16-first / DoubleRow-last so the DMA stream runs ahead of PE and
    the end-of-stream critical chain is the smallest group. The GEMM
    accumulates into one PSUM bank; lhsT = strided scan-output views;
    the SW scale rides through the relu chain (relu(s*x) = s*relu(x))
    and is undone by shipping Wo/SW. a-phases pipeline three js ahead.
  * Tail: PE-transpose h1, then 4 x [512,512] + [97,512] in transposed
    (feature-major) layout; per-m [128,128] PSUM tiles drain alternately
    on ScalarE/VectorE so the next layer's k-matmuls chase the drains;
    the output head runs in batch halves on twin PSUM banks. Biases enter
    PSUM via rank-1 matmuls (b1/b2 pre-scaled by SW on host); kernel()
    detects all-zero biases at runtime and uses a specialized build that
    drops them from the critical tail (the general path remains for
    nonzero biases).
  * DMA order follows the critical path: dir-0 WFE + onehot first (the
    first a-matmul gates everything), W1 stream next, dir-1 onehot inside
    the loop at j=1, tail-only W2/Wo after the W1 stream. a-phase PSUM is
    a 7-deep ring + 1 bank for the W1 accumulator; the a-ring is released
    after the GEMM so the tail pools (2+4+2 banks) fit in the 8 banks.
"""

import numpy as np
import ml_dtypes

F16 = np.float16
E3M4 = ml_dtypes.float8_e3m4
F8E4 = ml_dtypes.float8_e4m3
SW = 256.0            # W1 pre-scale; psum is SW-scaled, undone in the head
NT8 = 20              # t < NT8 ships as fp8e3 (RNN states are small early on)

MOD = 97
HID = 512
T = 32
B = 1024
NCORES = 8
BL = B // NCORES          # 128 batch per core
CL = T + 1                # chain length incl. separator column
FREE = BL * CL            # 4224 scan columns per tile
NEG = -60000.0
W1_GRP = 64               # W1 DMA groups of 4 k-chunks (512KB each)

_CACHE: dict = {}


def _build(nobias=False, reps=1):
    import concourse.tile as tile
    from concourse import bacc, mybir

    fp32 = mybir.dt.float32
    fp16 = mybir.dt.float16
    fp8 = mybir.dt.float8e3
    fp8e4 = mybir.dt.float8e4

    nc = bacc.Bacc(
        "TRN2", target_bir_lowering=False, debug=False, num_devices=NCORES
    )

    d = {
        "IDA": nc.dram_tensor("IDA", [128, 129], fp16, kind="ExternalInput").ap(),
        "WFE": nc.dram_tensor("WFE", [MOD, 2 * HID], fp16, kind="ExternalInput").ap(),
        "W1D": nc.dram_tensor("W1D", [8, 128, 2048], fp8e4, kind="ExternalInput").ap(),
        "W1A": nc.dram_tensor("W1A", [W1_GRP // 2, 128, 2048], fp8, kind="ExternalInput").ap(),
        "W1B": nc.dram_tensor("W1B", [W1_GRP * 3 // 8, 128, 2048], fp16, kind="ExternalInput").ap(),
        "W2O": nc.dram_tensor("W2O", [128, 4 * 512 + 4 * MOD], fp16, kind="ExternalInput").ap(),
        "BIA": nc.dram_tensor("BIA", [1, 1121], fp16, kind="ExternalInput").ap(),
        "OHX": nc.dram_tensor("OHX", [MOD, 2 * BL * T], fp8, kind="ExternalInput").ap(),
        "OUT": nc.dram_tensor("OUT", [MOD, BL], fp32, kind="ExternalOutput").ap(),
    }

    with tile.TileContext(nc) as tc:
        for _ in range(reps):
            _emit(tc, d, mybir, nobias)

    nc.compile()
    return nc


def _emit(tc, d, mybir, nobias=False):
    nc = tc.nc
    fp32 = mybir.dt.float32
    fp16 = mybir.dt.float16
    fp8 = mybir.dt.float8e3
    fp8e4 = mybir.dt.float8e4
    AF = mybir.ActivationFunctionType
    ALU = mybir.AluOpType

    from contextlib import ExitStack

    with ExitStack() as ctx:
        const = ctx.enter_context(tc.tile_pool(name="const", bufs=1))
        a_pool = ctx.enter_context(tc.tile_pool(name="apool", bufs=3))
        h_pool = ctx.enter_context(tc.tile_pool(name="hpool", bufs=4))
        w1_pool = ctx.enter_context(tc.tile_pool(name="w1pool", bufs=24))
        hp_pool = ctx.enter_context(tc.tile_pool(name="hppool", bufs=3))
        x8_pool = ctx.enter_context(tc.tile_pool(name="x8pool", bufs=4))
        h1_ctx = ExitStack()
        ps_h1 = h1_ctx.enter_context(tc.tile_pool(name="psh1", bufs=1, space="PSUM"))
        a_ctx = ExitStack()
        ps_a = a_ctx.enter_context(tc.tile_pool(name="psa", bufs=7, space="PSUM"))

        # ---- head ----
        # HWDGE descriptor generations serialize (~0.65us each), so DMA
        # instruction order IS the critical path: dir-0 WFE + onehot feed
        # the first a-phase, then the W1 stream; dir-1 onehot ships inside
        # the loop (first needed at j=4)
        ohall = const.tile([MOD, 2 * BL * T], fp8)
        ohsb = [ohall[:, 0:BL * T], ohall[:, BL * T:2 * BL * T]]
        wfe = const.tile([MOD, 2 * HID], fp16)
        # lead with one big W1 group: its 1.4us transfer hides the
        # desc-gens of the small critical-path DMAs behind it, closing the
        # early DMA-pool gaps (the stream, not the head compute, is the
        # end-to-end bound)
        w1_pre = {}
        w_t0 = w1_pool.tile([128, 2048], fp16, tag="w_t")
        nc.sync.dma_start(w_t0[:], d["W1B"][0])
        w1_pre[0] = w_t0
        nc.sync.dma_start(wfe[:, 0:HID], d["WFE"][:, 0:HID])
        nc.sync.dma_start(ohsb[0][:, 0:1024], d["OHX"][:, 0:1024])
        nc.sync.dma_start(ohsb[0][:, 1024:4096], d["OHX"][:, 1024:4096])
        if not nobias:
            bia = const.tile([1, 1121], fp16)
            nc.sync.dma_start(bia[:], d["BIA"])
            b1sb = bia[:, 0:512]
            b2r = bia[:, 512:1024]
            bor = bia[:, 1024:1121]
        nc.sync.dma_start(wfe[:, HID:2 * HID], d["WFE"][:, HID:2 * HID])
        w_t1 = w1_pool.tile([128, 2048], fp16, tag="w_t")
        nc.sync.dma_start(w_t1[:], d["W1B"][1])
        w1_pre[1] = w_t1
        ida = const.tile([128, 129], fp16)
        nc.sync.dma_start(ida[:], d["IDA"])
        idsb = ida[:, 0:128]
        w2o = const.tile([128, 4 * 512 + 4 * MOD], fp16)
        w2sb = w2o[:, 0:2048]
        wosb = w2o[:, 2048:2048 + 4 * MOD]
        ones = const.tile([1, 128], fp16)
        nc.vector.memset(ones[:], 1.0)
        zero = const.tile([128, 1], fp16)
        nc.vector.memset(zero[:], 0.0)

        # ---- drive terms + scans + linear1, interleaved per j = dir*4 + m ----
        # a = WfeB @ onehot in 8 PSUM blocks of 16 chains; ScalarE lays each
        # block into the scan layout [p, b*33 + s]; the DVE scan computes
        # h = relu(a + h_prev) for all 128 chains in one instruction; then
        # the two W1 groups for this j stream in and accumulate into psum_h1.
        # W1 group order is (dir, m)-major so group G only needs scan j = G//8.
        psum_h1 = ps_h1.tile([128, 512], fp32)
        bias_done = [False]

        def a_phase(j):
            dd, m = j // 4, j % 4
            a_sb = a_pool.tile([128, FREE], fp16, tag="a")
            sep = a_sb[:].rearrange("p (b t) -> p b t", t=CL)[:, :, T]
            nc.vector.memset(sep, NEG)
            lhsT = wfe[:, dd * HID + m * 128: dd * HID + m * 128 + 128]
            for q in range(8):
                pa = ps_a.tile([128, 512], fp32, tag="pa")
                nc.tensor.matmul(
                    pa[:], lhsT, ohsb[dd][:, q * 512:(q + 1) * 512],
                    start=True, stop=True,
                )
                av = a_sb[:].rearrange("p (b t) -> p b t", t=CL)[:, 16 * q:16 * q + 16, 0:T]
                pv = pa[:].rearrange("p (b t) -> p b t", t=T)
                nc.scalar.copy(av, pv)
            h_t = h_pool.tile([128, FREE], fp16, tag="h")
            if j == 0:
                # head-critical: scan in q-pair quarters, chasing the copies
                # (q<4 copies split Act/DVE; q>=4 whole on ScalarE so
                # VectorE can scan continuously)
                QC = 32 * CL
                for i in range(4):
                    nc.vector.tensor_tensor_scan(
                        h_t[:, i * QC:(i + 1) * QC], a_sb[:, i * QC:(i + 1) * QC],
                        zero[:].broadcast_to([128, QC]),
                        initial=0.0, op0=ALU.add, op1=ALU.max,
                    )
            else:
                nc.vector.tensor_tensor_scan(
                    h_t[:], a_sb[:], zero[:].broadcast_to([128, FREE]),
                    initial=0.0, op0=ALU.add, op1=ALU.max,
                )
            # e4m3 copy of the t<4 slice: both DoubleRow operands must be
            # fp8e4; the early hidden states are small so the quantization
            # noise is cheap (err ~ h_t * dW)
            x8 = x8_pool.tile([128, 512], fp8e4, tag="x8")
            nc.scalar.copy(
                x8[:].rearrange("p (t b) -> p t b", t=4),
                h_t[:].rearrange("p (b t) -> p t b", t=CL)[:, 0:4, :],
            )
            return h_t, x8

        # per-j group order: big fp16 groups first so the DMA stream runs
        # ahead of PE consumption (the fp8 groups at the end need half the
        # bytes per PE-cycle); tg0 (DoubleRow) at slot 2 so its e4m3
        # operand copy hides behind the first two groups
        TG_ORDER = (5, 6, 1, 7, 2, 3, 4, 0)
        hs = {0: a_phase(0), 1: a_phase(1), 2: a_phase(2)}
        for j in range(8):
            h_t, x8 = hs[j]
            hv = h_t[:].rearrange("p (b t) -> p t b", t=CL)
            for gi, tg in enumerate(TG_ORDER):
                flat = j * 8 + gi
                if tg == 0:
                    src_ap, wdt = d["W1D"][j], fp8e4
                elif tg <= 4:
                    src_ap, wdt = d["W1A"][j * 4 + tg - 1], fp8
                else:
                    src_ap, wdt = d["W1B"][j * 3 + tg - 5], fp16
                w_t = w1_pre.pop(flat, None)
                if w_t is None:
                    w_t = w1_pool.tile([128, 2048], wdt, tag="w_t")
                    nc.sync.dma_start(w_t[:], src_ap)
                last_grp = flat == W1_GRP - 1
                opener = False
                if not bias_done[0]:
                    if nobias:
                        # all biases are zero (checked at runtime in
                        # kernel()): the first W1 matmul opens the group
                        opener = True
                    else:
                        # rank-1 bias opens the accumulation: ones.T @ b1
                        # broadcasts b1 over the batch partitions
                        nc.tensor.matmul(psum_h1[:], ones[:], b1sb,
                                         start=True, stop=False)
                    bias_done[0] = True
                if tg == 0:
                    # DoubleRow: contracts two k-tiles per instruction at
                    # 0.5 cycles/col; group layout [p, pair, slot, n]
                    wv = w_t[:].rearrange("p (pr sl n) -> p pr sl n", pr=2, sl=2)
                    x8v = x8[:].rearrange("p (t b) -> p t b", t=4)
                    for pair in range(2):
                        for nh in range(2):
                            nc.tensor.matmul(
                                psum_h1[:, nh * 256:(nh + 1) * 256],
                                x8v[:, 2 * pair:2 * pair + 2, :],
                                wv[:, pair, :, nh * 256:(nh + 1) * 256],
                                start=False,
                                stop=(last_grp and pair == 1),
                                perf_mode=mybir.MatmulPerfMode.DoubleRow,
                            )
                else:
                    for c in range(4):
                        t_idx = tg * 4 + c
                        nc.tensor.matmul(
                            psum_h1[:], hv[:, t_idx, :], w_t[:, c * 512:(c + 1) * 512],
                            start=(opener and c == 0), stop=(last_grp and c == 3),
                        )
                if gi == 0 and j + 3 < 8:
                    if j == 1:
                        # dir-1 onehot: first needed by a_phase(4), emitted
                        # at the top of j=1; keeps it off the head path
                        nc.sync.dma_start(ohsb[1], d["OHX"][:, BL * T:2 * BL * T])
                    hs[j + 3] = a_phase(j + 3)
        # tail-only weights ship after the W1 stream so the last W1 byte
        # (the critical one) arrives ~1.7us earlier; this DMA overlaps the
        # h1 drain + transposes and lands before the first layer matmul
        nc.sync.dma_start(w2o[:], d["W2O"][:])
        a_ctx.close()  # release the a-phase PSUM banks for the tail pools
        h1sb = const.tile([128, 512], fp16)
        nc.scalar.activation(h1sb[:], psum_h1[:], AF.Relu)
        h1_ctx.close()
        ps_t = ctx.enter_context(tc.tile_pool(name="pst", bufs=1, space="PSUM"))
        ps_l = ctx.enter_context(tc.tile_pool(name="psl", bufs=4, space="PSUM"))
        ps_o = ctx.enter_context(tc.tile_pool(name="pso", bufs=2, space="PSUM"))

        # ---- transpose h1 to feature-major [512, 128] ----
        # Twin PSUM banks: ScalarE drains one while VectorE drains the other
        # (Tile serializes same-bank readers)
        pt_a = ps_t.tile([128, 256], fp16, tag="pta")
        pt_b = ps_t.tile([128, 256], fp16, tag="ptb")
        cur = hp_pool.tile([128, 512], fp16, tag="hp")
        for m in (0, 1):
            nc.tensor.transpose(
                pt_a[:, (m % 2) * 128:(m % 2) * 128 + 128],
                h1sb[:, m * 128:(m + 1) * 128], idsb[:])
        nc.scalar.copy(cur[:, 0:256], pt_a[:])
        for m in (2, 3):
            nc.tensor.transpose(
                pt_b[:, (m % 2) * 128:(m % 2) * 128 + 128],
                h1sb[:, m * 128:(m + 1) * 128], idsb[:])
        nc.vector.tensor_copy(cur[:, 256:512], pt_b[:])

        # ---- 4 x (h = relu(W2 @ h' + b2)), feature-major, col block = m ----
        # per-m psum tiles so each 128-col block drains (Act/DVE alternating)
        # as soon as its 4 k-matmuls finish, and the next layer's k-matmuls
        # chase the drains instead of waiting for the full 512
        for _L in range(4):
            hq = hp_pool.tile([128, 512], fp16, tag="hp")
            for m in range(4):
                pl = ps_l.tile([128, 128], fp32, tag="pl")
                if not nobias:
                    nc.tensor.matmul(
                        pl[:], b2r[:, m * 128:(m + 1) * 128], ones[:],
                        start=True, stop=False,
                    )
                for k in range(4):
                    nc.tensor.matmul(
                        pl[:],
                        w2sb[:, k * 512 + m * 128: k * 512 + m * 128 + 128],
                        cur[:, k * 128:(k + 1) * 128],
                        start=(nobias and k == 0), stop=(k == 3),
                    )
                hsl = hq[:, m * 128:(m + 1) * 128]
                if m % 2 == 0:
                    nc.scalar.activation(hsl, pl[:], AF.Relu)
                else:
                    nc.vector.tensor_scalar_max(hsl, pl[:], 0.0)
            cur = hq

        # ---- output head: out' = Wo @ h' + bo  -> [97, 128] ----
        # batch halves on separate banks so the two drains run in parallel
        osb = const.tile([MOD, BL], fp32)
        for hh in range(2):
            po = ps_o.tile([MOD, 64], fp32, tag="po")
            if not nobias:
                nc.tensor.matmul(po[:], bor, ones[:, 0:64], start=True, stop=False)
            for k in range(4):
                nc.tensor.matmul(
                    po[:], wosb[:, k * MOD:(k + 1) * MOD],
                    cur[:, k * 128 + hh * 64: k * 128 + hh * 64 + 64],
                    start=(nobias and k == 0), stop=(k == 3),
                )
            if hh == 0:
                nc.scalar.copy(osb[:, 0:64], po[:])
            else:
                nc.vector.tensor_copy(osb[:, 64:128], po[:])
        nc.sync.dma_start(d["OUT"], osb[:])


def _host_prep(inputs):
    x = np.asarray(inputs["x"]).astype(np.int64)          # [B, T]
    emb = np.asarray(inputs["emb"], np.float32)           # [97, 512]
    Wf = np.asarray(inputs["Wf"], np.float32)
    bf = np.asarray(inputs["bf"], np.float32)
    Wb = np.asarray(inputs["Wb"], np.float32)
    bb = np.asarray(inputs["bb"], np.float32)
    W1 = np.asarray(inputs["W1"], np.float32)             # [512, 32768]
    b1 = np.asarray(inputs["b1"], np.float32)
    W2 = np.asarray(inputs["W2"], np.float32)
    b2 = np.asarray(inputs["b2"], np.float32)
    Wo = np.asarray(inputs["Wo"], np.float32)             # [97, 512]
    bo = np.asarray(inputs["bo"], np.float32)

    # fold embedding gather + input projection + bias:
    # a_d[:, b, s] = (Wd @ emb.T + bd)[:, idx] since onehot has exactly one 1
    WFE = np.ascontiguousarray(np.stack([
        (Wf @ emb.T + bf[:, None]).T,                     # [97, 512]
        (Wb @ emb.T + bb[:, None]).T,
    ]).transpose(1, 0, 2).reshape(MOD, 2 * HID)).astype(F16)

    # per-core one-hot of x, col = b*32 + s; fwd s = t, bwd s = reversed t.
    # Pure reformatting of the index tensor; 0/1 are exact in fp8e3.
    xc = x.reshape(NCORES, BL, T)
    XR = np.concatenate([
        xc.reshape(NCORES, BL * T), xc[:, :, ::-1].reshape(NCORES, BL * T)
    ], axis=1)                                            # [NC, 8192] int
    OHX = (XR[:, None, :] == np.arange(MOD)[None, :, None]).astype(E3M4)
    IDA = np.concatenate([
        np.eye(128, dtype=np.float32),
        np.arange(128, dtype=np.float32).reshape(128, 1),
    ], axis=1).astype(F16)

    # W1 -> [64, 128, 2048]: group G = (d, m, tg) holds k-chunks for
    # t = 4*tg .. 4*tg+3 of direction d, hid-tile m, side by side
    # W1.T row layout is [t, d, m, p]-major (xcat col = t*1024 + d*512 + m*128)
    # Everything is pre-scaled by SW; tg < 4 (t < 16, where the RNN states are
    # still small) ships as fp8e3, the rest as fp16. The psum is SW-scaled,
    # undone by shipping Wo/SW.
    W1S = (
        W1.T.reshape(8, 4, 2, 4, 128, 512)       # [tg, tc, d, m, p, col]
        .transpose(2, 3, 0, 4, 1, 5)             # [d, m, tg, p, tc, col]
        .reshape(W1_GRP, 128, 2048)
    ) * SW
    tgm = np.arange(W1_GRP) % 8
    W1D = np.ascontiguousarray(W1S[tgm == 0]).astype(F8E4)            # [8, ...] DoubleRow
    W1A = np.ascontiguousarray(W1S[(tgm >= 1) & (tgm <= 4)]).astype(E3M4)  # [32, ...]
    W1B = np.ascontiguousarray(W1S[tgm >= 5]).astype(F16)             # [24, ...]
    W2S = np.ascontiguousarray(W2.T.reshape(4, 128, 512).transpose(1, 0, 2).reshape(128, 2048)).astype(F16)
    WOS = np.ascontiguousarray((Wo.T / SW).reshape(4, 128, MOD).transpose(1, 0, 2).reshape(128, 4 * MOD)).astype(F16)
    W2O = np.concatenate([W2S, WOS], axis=1)
    BIAH = np.concatenate([b1 * SW, b2 * SW, bo]).astype(F16).reshape(1, -1)  # [1, 1121]

    shared = {"WFE": WFE, "W1D": W1D, "W1A": W1A, "W1B": W1B, "W2O": W2O, "IDA": IDA, "BIA": BIAH}
    in_maps = [dict(shared, OHX=OHX[c]) for c in range(NCORES)]
    return in_maps


def _get_nc(nobias=True):
    key = ("nc", nobias)
    if key not in _CACHE:
        _CACHE[key] = _build(nobias)
    return _CACHE[key]


def kernel(**inputs):
    from concourse.bass_utils import run_bass_kernel_spmd

    nobias = all(
        not np.any(np.asarray(inputs[k], np.float32))
        for k in ("bf", "bb", "b1", "b2", "bo")
    )
    nc = _get_nc(nobias)
    in_maps = _host_prep(inputs)
    res = run_bass_kernel_spmd(nc, in_maps, list(range(NCORES)))
    outs = [np.asarray(res.results[c]["OUT"], np.float32) for c in range(NCORES)]
    return np.ascontiguousarray(np.concatenate([o.T for o in outs], axis=0))  # [1024, 97]

